# revision 21
# baseline (speedup 1.0000x reference)
"""Trainium2 Bass kernel for nn_DIE: per-pixel channel SE gate.

    h    = relu(W1 @ x[:, :, i, j])      # [B, 32, H, W]
    gate = sigmoid(W2 @ h)               # [B, 512, H, W]
    y    = gate * x

Sharding: pure data parallel over the batch dim (B=8 -> 8 cores).
Each core streams its [512, 192*192] slab through SBUF in DMA tiles
of 2048 pixels (8KB contiguous runs per channel). Matmuls run in
float32r mode (fp32 bits, single-pass PE streaming at 1 row/cycle
for moving dim >= 256, ~4x faster than plain fp32 matmul; ~1.9e-4
absmax rounding error in the gate). The final elementwise multiply
uses the fp32 x exactly.
"""

import sys

for _p in ("/opt/trn_rl_repo",):
    if _p not in sys.path:
        sys.path.insert(0, _p)

import numpy as np

import concourse.bacc as bacc
import concourse.bass as bass
import concourse.mybir as mybir
from concourse import tile
from concourse.bass_utils import run_bass_kernel_spmd

B, C, H, W = 8, 512, 192, 192
R = 32            # C // RED
NPIX = H * W      # 36864 pixels per batch element
N_CORES = 8
DMA_N = 2048      # pixels per DMA tile
SUB_N = 512       # pixels per compute sub-tile (one PSUM bank of fp32)
PART = 128
G = C // PART     # 4 channel groups

F32 = mybir.dt.float32
F32R = mybir.dt.float32r
AF = mybir.ActivationFunctionType


def build(npix: int = NPIX, dma_n: int = DMA_N):
    """Build the per-core Bass program (SPMD: identical on all cores)."""
    assert npix % dma_n == 0 and dma_n % SUB_N == 0
    # Small tiles at the head (prime the out-DMA stream sooner: the out
    # ring is the critical path and can only start after the first tile's
    # compute) and at the tail (let the final transfers drain sooner).
    if npix > 4 * dma_n and dma_n > SUB_N:
        ramp = dma_n // SUB_N  # e.g. two 512s replace one 1024 tile
        tile_sizes = (
            [SUB_N] * ramp
            + [dma_n] * (npix // dma_n - 2)
            + [SUB_N] * ramp
        )
    else:
        tile_sizes = [dma_n] * (npix // dma_n)
    assert sum(tile_sizes) == npix

    nc = bacc.Bacc("TRN2", target_bir_lowering=False, debug=False, num_devices=N_CORES)

    # float32r carries plain fp32 bits; declaring the DRAM side f32r makes
    # the DMA the "rounding producer" the BIR verifier requires for f32r
    # matmul operands.
    x_d = nc.dram_tensor("x", [C, npix], F32R, kind="ExternalInput").ap()
    w1t_d = nc.dram_tensor("w1t", [C, R], F32R, kind="ExternalInput").ap()  # W1.T
    w2t_d = nc.dram_tensor("w2t", [R, C], F32R, kind="ExternalInput").ap()  # W2.T
    y_d = nc.dram_tensor("y", [C, npix], F32, kind="ExternalOutput").ap()

    with tile.TileContext(nc) as tc:
        with (
            tc.tile_pool(name="wpool", bufs=1) as wpool,
            tc.tile_pool(name="xp", bufs=3) as xp,
            tc.tile_pool(name="hp", bufs=4) as hp,
            tc.tile_pool(name="gp", bufs=8) as gp,
            tc.tile_pool(name="op", bufs=2) as op_,
            tc.tile_pool(name="hpsum", bufs=2, space=bass.MemorySpace.PSUM) as hpsum,
            tc.tile_pool(name="gpsum", bufs=4, space=bass.MemorySpace.PSUM) as gpsum,
        ):
            # Weights, loaded once.
            # w1t[p, g, r] = W1T[g*128+p, r]; w2t[p, g, m] = W2T[p, g*128+m].
            w1t = wpool.tile([PART, G, R], F32R)
            nc.sync.dma_start(w1t[:], w1t_d.rearrange("(g p) r -> p g r", p=PART))
            w2t = wpool.tile([R, G, PART], F32R)
            nc.sync.dma_start(w2t[:], w2t_d.rearrange("r (g m) -> r g m", m=PART))

            n0 = 0
            for tn in tile_sizes:
                xt = xp.tile([PART, G, tn], F32R, tag="xt")
                nc.sync.dma_start(
                    xt[:],
                    x_d[:, n0 : n0 + tn].rearrange("(g p) n -> p g n", p=PART),
                )

                ot = op_.tile([PART, G, tn], F32, tag="ot")
                for s in range(tn // SUB_N):
                    sl = slice(s * SUB_N, (s + 1) * SUB_N)
                    # h[r, n] = sum_c W1[r, c] x[c, n], accumulated over chunks
                    hps = hpsum.tile([R, SUB_N], F32, tag="hps")
                    for g in range(G):
                        nc.tensor.matmul(
                            hps[:], w1t[:, g, :], xt[:, g, sl],
                            start=(g == 0), stop=(g == G - 1),
                        )
                    hs = hp.tile([R, SUB_N], F32R, tag="hs")
                    nc.scalar.activation(hs[:], hps[:], AF.Relu)

                    for g in range(G):
                        gps = gpsum.tile([PART, SUB_N], F32, tag="gps")
                        nc.tensor.matmul(
                            gps[:], w2t[:, g, :], hs[:], start=True, stop=True
                        )
                        gs = gp.tile([PART, SUB_N], F32, tag="gs")
                        nc.scalar.activation(gs[:], gps[:], AF.Sigmoid)
                        nc.vector.tensor_mul(
                            ot[:, g, sl], gs[:], xt[:, g, sl].bitcast(F32)
                        )

                nc.scalar.dma_start(
                    y_d[:, n0 : n0 + tn].rearrange("(g p) n -> p g n", p=PART),
                    ot[:],
                )
                n0 += tn

    nc.compile()
    return nc


def _plausible(y: np.ndarray, x: np.ndarray) -> bool:
    """Cheap integrity check: y = sigmoid(.)*x implies |y| <= |x| (modulo
    a ulp of rounding), finite everywhere, and y is never 0 where x isn't
    tiny (the gate can't underflow for this weight scale). Transient DMA
    corruption / stale pages violate these with near-certainty."""
    if not np.isfinite(y).all():
        return False
    ax = np.abs(x)
    if (np.abs(y) > ax * 1.00001 + 1e-30).any():
        return False
    if np.count_nonzero((y == 0.0) & (ax > 1e-3)) > y.size // 1_000_000:
        return False
    return True


def kernel(x: np.ndarray, W1: np.ndarray, W2: np.ndarray, **run_kwargs):
    """Full-input entry point: shards batch over 8 cores, returns full output."""
    x = np.asarray(x)
    assert x.shape == (B, C, H, W), x.shape
    nc = build()

    w1t = np.ascontiguousarray(np.asarray(W1).T.astype(np.float32))  # [512, 32]
    w2t = np.ascontiguousarray(np.asarray(W2).T.astype(np.float32))  # [32, 512]
    in_maps = [
        {
            "x": np.ascontiguousarray(x[i].reshape(C, NPIX).astype(np.float32)),
            "w1t": w1t,
            "w2t": w2t,
        }
        for i in range(N_CORES)
    ]
    retries = 2 if not run_kwargs.get("trace") else 0
    for attempt in range(retries + 1):
        res = run_bass_kernel_spmd(nc, in_maps, list(range(N_CORES)), **run_kwargs)
        if all(
            _plausible(res.results[i]["y"], in_maps[i]["x"]) for i in range(N_CORES)
        ):
            break
    y = np.stack([res.results[i]["y"].reshape(C, H, W) for i in range(N_CORES)])
    if run_kwargs:
        return y, res
    return y


# revision 22
# speedup vs baseline: 1.0258x; 1.0258x over previous
"""Trainium2 Bass kernel for nn_DIE: per-pixel channel SE gate.

    h    = relu(W1 @ x[:, :, i, j])      # [B, 32, H, W]
    gate = sigmoid(W2 @ h)               # [B, 512, H, W]
    y    = gate * x

Sharding: pure data parallel over the batch dim (B=8 -> 8 cores).
Each core streams its [512, 192*192] slab through SBUF in DMA tiles
of 2048 pixels (8KB contiguous runs per channel). Matmuls run in
float32r mode (fp32 bits, single-pass PE streaming at 1 row/cycle
for moving dim >= 256, ~4x faster than plain fp32 matmul; ~1.9e-4
absmax rounding error in the gate). The final elementwise multiply
uses the fp32 x exactly.
"""

import sys

for _p in ("/opt/trn_rl_repo",):
    if _p not in sys.path:
        sys.path.insert(0, _p)

import numpy as np

import concourse.bacc as bacc
import concourse.bass as bass
import concourse.mybir as mybir
from concourse import tile
from concourse.bass_utils import run_bass_kernel_spmd

B, C, H, W = 8, 512, 192, 192
R = 32            # C // RED
NPIX = H * W      # 36864 pixels per batch element
N_CORES = 8
DMA_N = 2048      # pixels per DMA tile
SUB_N = 512       # pixels per compute sub-tile (one PSUM bank of fp32)
PART = 128
G = C // PART     # 4 channel groups

F32 = mybir.dt.float32
F32R = mybir.dt.float32r
AF = mybir.ActivationFunctionType


def build(npix: int = NPIX, dma_n: int = DMA_N):
    """Build the per-core Bass program (SPMD: identical on all cores)."""
    assert npix % dma_n == 0 and dma_n % SUB_N == 0
    # Small tiles at the head (prime the out-DMA stream sooner: the out
    # ring is the critical path and can only start after the first tile's
    # compute) and at the tail (let the final transfers drain sooner).
    if npix > 4 * dma_n and dma_n >= 2 * SUB_N:
        half = dma_n // 2  # two half-tiles each end: early out-start,
        # without the 2KB-run descriptor inefficiency of SUB_N tiles
        tile_sizes = (
            [half] * 2
            + [dma_n] * (npix // dma_n - 2)
            + [half] * 2
        )
    else:
        tile_sizes = [dma_n] * (npix // dma_n)
    assert sum(tile_sizes) == npix

    nc = bacc.Bacc("TRN2", target_bir_lowering=False, debug=False, num_devices=N_CORES)

    # float32r carries plain fp32 bits; declaring the DRAM side f32r makes
    # the DMA the "rounding producer" the BIR verifier requires for f32r
    # matmul operands.
    x_d = nc.dram_tensor("x", [C, npix], F32R, kind="ExternalInput").ap()
    w1t_d = nc.dram_tensor("w1t", [C, R], F32R, kind="ExternalInput").ap()  # W1.T
    w2t_d = nc.dram_tensor("w2t", [R, C], F32R, kind="ExternalInput").ap()  # W2.T
    y_d = nc.dram_tensor("y", [C, npix], F32, kind="ExternalOutput").ap()

    with tile.TileContext(nc) as tc:
        with (
            tc.tile_pool(name="wpool", bufs=1) as wpool,
            tc.tile_pool(name="xp", bufs=3) as xp,
            tc.tile_pool(name="hp", bufs=4) as hp,
            tc.tile_pool(name="gp", bufs=8) as gp,
            tc.tile_pool(name="op", bufs=2) as op_,
            tc.tile_pool(name="hpsum", bufs=2, space=bass.MemorySpace.PSUM) as hpsum,
            tc.tile_pool(name="gpsum", bufs=4, space=bass.MemorySpace.PSUM) as gpsum,
        ):
            # Weights, loaded once.
            # w1t[p, g, r] = W1T[g*128+p, r]; w2t[p, g, m] = W2T[p, g*128+m].
            w1t = wpool.tile([PART, G, R], F32R)
            nc.sync.dma_start(w1t[:], w1t_d.rearrange("(g p) r -> p g r", p=PART))
            w2t = wpool.tile([R, G, PART], F32R)
            nc.sync.dma_start(w2t[:], w2t_d.rearrange("r (g m) -> r g m", m=PART))

            n0 = 0
            for tn in tile_sizes:
                xt = xp.tile([PART, G, tn], F32R, tag="xt")
                nc.sync.dma_start(
                    xt[:],
                    x_d[:, n0 : n0 + tn].rearrange("(g p) n -> p g n", p=PART),
                )

                ot = op_.tile([PART, G, tn], F32, tag="ot")
                for s in range(tn // SUB_N):
                    sl = slice(s * SUB_N, (s + 1) * SUB_N)
                    # h[r, n] = sum_c W1[r, c] x[c, n], accumulated over chunks
                    hps = hpsum.tile([R, SUB_N], F32, tag="hps")
                    for g in range(G):
                        nc.tensor.matmul(
                            hps[:], w1t[:, g, :], xt[:, g, sl],
                            start=(g == 0), stop=(g == G - 1),
                        )
                    hs = hp.tile([R, SUB_N], F32R, tag="hs")
                    nc.scalar.activation(hs[:], hps[:], AF.Relu)

                    for g in range(G):
                        gps = gpsum.tile([PART, SUB_N], F32, tag="gps")
                        nc.tensor.matmul(
                            gps[:], w2t[:, g, :], hs[:], start=True, stop=True
                        )
                        gs = gp.tile([PART, SUB_N], F32, tag="gs")
                        nc.scalar.activation(gs[:], gps[:], AF.Sigmoid)
                        nc.vector.tensor_mul(
                            ot[:, g, sl], gs[:], xt[:, g, sl].bitcast(F32)
                        )

                nc.scalar.dma_start(
                    y_d[:, n0 : n0 + tn].rearrange("(g p) n -> p g n", p=PART),
                    ot[:],
                )
                n0 += tn

    nc.compile()
    return nc


def _plausible(y: np.ndarray, x: np.ndarray) -> bool:
    """Cheap integrity check: y = sigmoid(.)*x implies |y| <= |x| (modulo
    a ulp of rounding), finite everywhere, and y is never 0 where x isn't
    tiny (the gate can't underflow for this weight scale). Transient DMA
    corruption / stale pages violate these with near-certainty."""
    if not np.isfinite(y).all():
        return False
    ax = np.abs(x)
    if (np.abs(y) > ax * 1.00001 + 1e-30).any():
        return False
    if np.count_nonzero((y == 0.0) & (ax > 1e-3)) > y.size // 1_000_000:
        return False
    return True


def kernel(x: np.ndarray, W1: np.ndarray, W2: np.ndarray, **run_kwargs):
    """Full-input entry point: shards batch over 8 cores, returns full output."""
    x = np.asarray(x)
    assert x.shape == (B, C, H, W), x.shape
    nc = build()

    w1t = np.ascontiguousarray(np.asarray(W1).T.astype(np.float32))  # [512, 32]
    w2t = np.ascontiguousarray(np.asarray(W2).T.astype(np.float32))  # [32, 512]
    in_maps = [
        {
            "x": np.ascontiguousarray(x[i].reshape(C, NPIX).astype(np.float32)),
            "w1t": w1t,
            "w2t": w2t,
        }
        for i in range(N_CORES)
    ]
    retries = 2 if not run_kwargs.get("trace") else 0
    for attempt in range(retries + 1):
        res = run_bass_kernel_spmd(nc, in_maps, list(range(N_CORES)), **run_kwargs)
        if all(
            _plausible(res.results[i]["y"], in_maps[i]["x"]) for i in range(N_CORES)
        ):
            break
    y = np.stack([res.results[i]["y"].reshape(C, H, W) for i in range(N_CORES)])
    if run_kwargs:
        return y, res
    return y


# revision 24
# speedup vs baseline: 1.1721x; 1.1426x over previous
"""Trainium2 Bass kernel for nn_DIE: per-pixel channel SE gate.

    h    = relu(W1 @ x[:, :, i, j])      # [B, 32, H, W]
    gate = sigmoid(W2 @ h)               # [B, 512, H, W]
    y    = gate * x

Sharding: pure data parallel over the batch dim (B=8 -> 8 cores).
Each core streams its [512, 192*192] slab through SBUF in DMA tiles
of 2048 pixels (8KB contiguous runs per channel). Matmuls run in
float32r mode (fp32 bits, single-pass PE streaming at 1 row/cycle
for moving dim >= 256, ~4x faster than plain fp32 matmul; ~1.9e-4
absmax rounding error in the gate). The final elementwise multiply
uses the fp32 x exactly.
"""

import sys

for _p in ("/opt/trn_rl_repo",):
    if _p not in sys.path:
        sys.path.insert(0, _p)

import numpy as np

import concourse.bacc as bacc
import concourse.bass as bass
import concourse.mybir as mybir
from concourse import tile
from concourse.bass_utils import run_bass_kernel_spmd

B, C, H, W = 8, 512, 192, 192
R = 32            # C // RED
NPIX = H * W      # 36864 pixels per batch element
N_CORES = 8
DMA_N = 2048      # pixels per DMA tile
SUB_N = 512       # pixels per compute sub-tile (one PSUM bank of fp32)
PART = 128
G = C // PART     # 4 channel groups

F32 = mybir.dt.float32
F32R = mybir.dt.float32r
AF = mybir.ActivationFunctionType


def build(npix: int = NPIX, dma_n: int = DMA_N):
    """Build the per-core Bass program (SPMD: identical on all cores)."""
    assert npix % dma_n == 0 and dma_n % SUB_N == 0
    # Small tiles at the head (prime the out-DMA stream sooner: the out
    # ring is the critical path and can only start after the first tile's
    # compute) and at the tail (let the final transfers drain sooner).
    if npix > 4 * dma_n and dma_n >= 2 * SUB_N:
        half = dma_n // 2  # two half-tiles each end: early out-start,
        # without the 2KB-run descriptor inefficiency of SUB_N tiles
        tile_sizes = (
            [half] * 2
            + [dma_n] * (npix // dma_n - 2)
            + [half] * 2
        )
    else:
        tile_sizes = [dma_n] * (npix // dma_n)
    assert sum(tile_sizes) == npix

    nc = bacc.Bacc("TRN2", target_bir_lowering=False, debug=False, num_devices=N_CORES)

    # float32r carries plain fp32 bits; declaring the DRAM side f32r makes
    # the DMA the "rounding producer" the BIR verifier requires for f32r
    # matmul operands.
    x_d = nc.dram_tensor("x", [C, npix], F32R, kind="ExternalInput").ap()
    w1t_d = nc.dram_tensor("w1t", [C, R], F32R, kind="ExternalInput").ap()  # W1.T
    w2t_d = nc.dram_tensor("w2t", [R, C], F32R, kind="ExternalInput").ap()  # W2.T
    y_d = nc.dram_tensor("y", [C, npix], F32, kind="ExternalOutput").ap()

    with tile.TileContext(nc) as tc:
        with (
            tc.tile_pool(name="wpool", bufs=1) as wpool,
            tc.tile_pool(name="xp", bufs=3) as xp,
            tc.tile_pool(name="hp", bufs=4) as hp,
            tc.tile_pool(name="gp", bufs=8) as gp,
            tc.tile_pool(name="op", bufs=2) as op_,
            tc.tile_pool(name="hpsum", bufs=2, space=bass.MemorySpace.PSUM) as hpsum,
            tc.tile_pool(name="gpsum", bufs=4, space=bass.MemorySpace.PSUM) as gpsum,
        ):
            # Weights, loaded once.
            # w1t[p, g, r] = W1T[g*128+p, r]; w2t[p, g, m] = W2T[p, g*128+m].
            w1t = wpool.tile([PART, G, R], F32R)
            nc.sync.dma_start(w1t[:], w1t_d.rearrange("(g p) r -> p g r", p=PART))
            w2t = wpool.tile([R, G, PART], F32R)
            nc.sync.dma_start(w2t[:], w2t_d.rearrange("r (g m) -> r g m", m=PART))

            n0 = 0
            for ti, tn in enumerate(tile_sizes):
                xt = xp.tile([PART, G, tn], F32R, tag="xt")
                nc.sync.dma_start(
                    xt[:],
                    x_d[:, n0 : n0 + tn].rearrange("(g p) n -> p g n", p=PART),
                )

                ot = op_.tile([PART, G, tn], F32, tag="ot")
                for s in range(tn // SUB_N):
                    sl = slice(s * SUB_N, (s + 1) * SUB_N)
                    # h[r, n] = sum_c W1[r, c] x[c, n], accumulated over chunks
                    hps = hpsum.tile([R, SUB_N], F32, tag="hps")
                    for g in range(G):
                        nc.tensor.matmul(
                            hps[:], w1t[:, g, :], xt[:, g, sl],
                            start=(g == 0), stop=(g == G - 1),
                        )
                    hs = hp.tile([R, SUB_N], F32R, tag="hs")
                    nc.scalar.activation(hs[:], hps[:], AF.Relu)

                    for g in range(G):
                        gps = gpsum.tile([PART, SUB_N], F32, tag="gps")
                        nc.tensor.matmul(
                            gps[:], w2t[:, g, :], hs[:], start=True, stop=True
                        )
                        gs = gp.tile([PART, SUB_N], F32, tag="gs")
                        nc.scalar.activation(gs[:], gps[:], AF.Sigmoid)
                        nc.vector.tensor_mul(
                            ot[:, g, sl], gs[:], xt[:, g, sl].bitcast(F32)
                        )

                # Drain assist: the sync ring's loads finish before the tail
                # stores, so route the last few stores onto it — both HWDGE
                # rings then drain the output in parallel.
                out_eng = nc.sync if ti >= len(tile_sizes) - 3 else nc.scalar
                out_eng.dma_start(
                    y_d[:, n0 : n0 + tn].rearrange("(g p) n -> p g n", p=PART),
                    ot[:],
                )
                n0 += tn

    nc.compile()
    return nc


def _plausible(y: np.ndarray, x: np.ndarray) -> bool:
    """Cheap integrity check: y = sigmoid(.)*x implies |y| <= |x| (modulo
    a ulp of rounding), finite everywhere, and y is never 0 where x isn't
    tiny (the gate can't underflow for this weight scale). Transient DMA
    corruption / stale pages violate these with near-certainty."""
    if not np.isfinite(y).all():
        return False
    ax = np.abs(x)
    if (np.abs(y) > ax * 1.00001 + 1e-30).any():
        return False
    if np.count_nonzero((y == 0.0) & (ax > 1e-3)) > y.size // 1_000_000:
        return False
    return True


def kernel(x: np.ndarray, W1: np.ndarray, W2: np.ndarray, **run_kwargs):
    """Full-input entry point: shards batch over 8 cores, returns full output."""
    x = np.asarray(x)
    assert x.shape == (B, C, H, W), x.shape
    nc = build()

    w1t = np.ascontiguousarray(np.asarray(W1).T.astype(np.float32))  # [512, 32]
    w2t = np.ascontiguousarray(np.asarray(W2).T.astype(np.float32))  # [32, 512]
    in_maps = [
        {
            "x": np.ascontiguousarray(x[i].reshape(C, NPIX).astype(np.float32)),
            "w1t": w1t,
            "w2t": w2t,
        }
        for i in range(N_CORES)
    ]
    retries = 2 if not run_kwargs.get("trace") else 0
    for attempt in range(retries + 1):
        res = run_bass_kernel_spmd(nc, in_maps, list(range(N_CORES)), **run_kwargs)
        if all(
            _plausible(res.results[i]["y"], in_maps[i]["x"]) for i in range(N_CORES)
        ):
            break
    y = np.stack([res.results[i]["y"].reshape(C, H, W) for i in range(N_CORES)])
    if run_kwargs:
        return y, res
    return y
